# revision 21
# baseline (speedup 1.0000x reference)
"""Group-quantized linear (fake int4 per-group dequant) GEMV on 8 Trainium2 cores.

Reference computation (all fp32):
    qw = round_half_even(clip(W, -8, 7))            # W in [-8, 7) so clip is identity
    out = (qw.reshape(O, 64, 128) * scales[:, :, None]).reshape(O, O) @ x

Sharding: column-parallel — each core owns a 1024-row slice of W/scales,
x replicated, outputs concatenated (per the tensor-parallel hint).

Device pipeline, built around the HBM stream (memory-bound problem):
  DMA   : TWO HW DGE queues (SP + Activation engines) stream the weights
          concurrently (~400 GB/s aggregate vs ~310 single-queue).  The
          per-core weight slice is shipped pre-packed (pure host-side
          layout) into two linear regions, one per queue.  Bulk of the
          stream moves in 2 MiB "quarter" units (4 groups; 16 KiB
          partition-contiguous descriptors — measured fastest); the last
          8 groups move as 512 KiB per-group units so the compute tail
          trails the final bytes by ~2 us instead of ~12.
  DVE   : quantize via the fp32 magic-number trick (w + 1.5*2^23) -
          1.5*2^23 == round-half-even exactly, cast to bf16 (exact for
          ints in [-8, 7]); one tensor_scalar per unit.
  PE    : per (group g, out-chunk oc) matmul acc[:, oc, g, :2] =
          qw[128c, 128o].T @ x2[128c, 2] where x2 = [x_hi | x_lo] bf16
          Dekker split of x (fp32-accurate), all accumulated in one fp32
          PSUM tile [128, 8, 64, 2] (2 banks).
  DVE   : epilogue out[o] = sum_{g,j} acc[o, oc, g, j] * scales[o, oc, g]
          with hi/lo-duplicated scales: stage A (groups < 56) is emitted
          after the tail quantizes so DVE never blocks on PE mid-tail;
          stage B (groups 56..63) + combine are 3 tiny ops after the last
          matmul.  Scales ride mid-stream, split across both queues.
  PE/DVE: transpose [128, 8] result for a contiguous output DMA
"""

import numpy as np

IN_DIM = 8192
OUT_DIM = 8192
NUM_GROUPS = 64
GROUP_SIZE = 128  # IN_DIM // NUM_GROUPS
N_CORES = 8
PER_OUT = OUT_DIM // N_CORES  # 1024
P = 128
OC_N = PER_OUT // P  # 8

MAGIC = np.float32(12582912.0)  # 1.5 * 2**23: (w + MAGIC) - MAGIC == rint(w)

QUARTER = 4  # groups per DMA unit (16 KiB/partition descriptors)
N_QUARTERS = NUM_GROUPS // QUARTER  # 16
EP_SPLIT = 56  # epilogue stage-A covers groups [0, 56)

_cache = {}


def _units():
    """(kind, group_start, n_groups) in stream order; queue alternates.
    Uniform 2 MiB units: sub-1 MiB DMAs measured ~15% slower per queue
    (4 KiB descriptors) plus ~0.6 us inter-DMA gaps — a dense big-unit
    stream beats a fine-grained tail."""
    return [("q", qi * QUARTER, QUARTER) for qi in range(N_QUARTERS)]


def _split_multi_waits(nc):
    """walrus in this container accepts only ONE sync-wait per instruction;
    Tile's tail drain carries one per producer proc. Hoist extras onto
    same-engine NoOps placed immediately before — identical semantics for an
    in-order sequencer."""
    import concourse.mybir as mybir

    uid = 0
    for f in nc.m.functions:
        for blk in f.blocks:
            insts = blk.instructions
            if not any(
                i.sync_info is not None
                and i.sync_info.on_wait
                and len(i.sync_info.on_wait) > 1
                for i in insts
            ):
                continue
            new_insts = []
            for inst in insts:
                si = inst.sync_info
                if si is not None and si.on_wait and len(si.on_wait) > 1:
                    waits = list(si.on_wait)
                    for w in waits[:-1]:
                        uid += 1
                        new_insts.append(
                            mybir.InstNoOp(
                                name=f"I-waitsplit-{uid}",
                                engine=inst.engine,
                                ins=[],
                                outs=[],
                                sync_info=mybir.SyncInfo(on_wait=[w], on_update=[]),
                            )
                        )
                    inst.sync_info = mybir.SyncInfo(
                        on_wait=[waits[-1]], on_update=si.on_update
                    )
                new_insts.append(inst)
            blk.instructions = new_insts
    return nc


def build_nc(w_bufs=5, q_bufs=4, split_waits=True):
    import concourse.bass as bass
    import concourse.mybir as mybir
    import concourse.tile as tile
    from concourse.masks import make_identity

    f32 = mybir.dt.float32
    bf16 = mybir.dt.bfloat16
    add = mybir.AluOpType.add

    ng = NUM_GROUPS
    half_elems = IN_DIM * PER_OUT // 2

    nc = bass.Bass()
    wa_d = nc.dram_tensor("wa", [half_elems], f32, kind="ExternalInput")
    wb_d = nc.dram_tensor("wb", [half_elems], f32, kind="ExternalInput")
    x_d = nc.dram_tensor("x", [IN_DIM], f32, kind="ExternalInput")
    sc_d = nc.dram_tensor("scales", [P, OC_N, ng], f32, kind="ExternalInput")
    out_d = nc.dram_tensor("out", [PER_OUT], f32, kind="ExternalOutput")

    units = _units()

    with tile.TileContext(nc) as tc:
        with (
            tc.tile_pool(name="singles", bufs=1) as singles,
            tc.tile_pool(name="w", bufs=w_bufs) as wpool,
            tc.tile_pool(name="q", bufs=q_bufs) as qpool,
            tc.tile_pool(name="psum", bufs=1, space="PSUM") as psum,
        ):
            # ---- x load first on the SP queue (tiny), then weights flow.
            x_nat = singles.tile([ng, GROUP_SIZE], f32)
            nc.sync.dma_start(x_nat, x_d.rearrange("(g c) -> g c", c=GROUP_SIZE))

            # ---- weight stream: unit k on queue k%2 (A=SP, B=Act)
            sc_sb = singles.tile([P, OC_N, ng], f32)
            utiles = []
            offs = [0, 0]
            regions = [wa_d, wb_d]
            for k, (kind, gs, g) in enumerate(units):
                if gs == EP_SPLIT:
                    # scales ride just before the tail, half per queue —
                    # early enough for sc2 prep, late enough to not delay
                    # the pipeline fill
                    nc.sync.dma_start(
                        sc_sb[:, : OC_N // 2, :], sc_d[:, : OC_N // 2, :]
                    )
                    nc.scalar.dma_start(
                        sc_sb[:, OC_N // 2 :, :], sc_d[:, OC_N // 2 :, :]
                    )
                wf = wpool.tile(
                    [P, g, PER_OUT], f32, tag=f"wf_{kind}", name=f"wf{k}"
                )
                qe = k % 2
                eng = nc.sync if qe == 0 else nc.scalar
                n = P * g * PER_OUT
                eng.dma_start(
                    wf,
                    regions[qe][offs[qe] : offs[qe] + n].rearrange(
                        "(c g o) -> c g o", c=P, g=g
                    ),
                )
                offs[qe] += n
                utiles.append(wf)

            # ---- x prep: PE-transpose [ng,128] -> [128,ng], Dekker-split
            # into interleaved bf16 hi/lo [128, ng, 2].
            ident_g = singles.tile([ng, ng], f32)
            make_identity(nc, ident_g)
            ident_p = singles.tile([P, P], f32)
            make_identity(nc, ident_p)

            x_ps = psum.tile([P, ng], f32, tag="paux")
            nc.tensor.transpose(x_ps, x_nat, ident_g)
            xT = singles.tile([P, ng], f32)
            nc.vector.tensor_copy(out=xT, in_=x_ps)
            xhi = singles.tile([P, ng], bf16)
            nc.vector.tensor_copy(out=xhi, in_=xT)
            xhi32 = singles.tile([P, ng], f32)
            nc.vector.tensor_copy(out=xhi32, in_=xhi)
            xlo32 = singles.tile([P, ng], f32)
            nc.vector.tensor_tensor(xlo32, xT, xhi32, mybir.AluOpType.subtract)
            x2 = singles.tile([P, ng, 2], bf16)
            nc.vector.tensor_copy(out=x2[:, :, 0], in_=xhi)
            nc.vector.tensor_copy(out=x2[:, :, 1], in_=xlo32)

            # one fused PSUM accumulator [128, oc, g, hi/lo] (2 banks)
            acc = psum.tile([P, OC_N, ng, 2], f32, tag="pacc")
            accf = acc.rearrange("p oc g j -> p oc (g j)")

            # sc2 = scales duplicated over hi/lo, for flat epilogue APs
            sc2 = singles.tile([P, OC_N, ng, 2], f32)

            # ---- main loop: per-unit quantize + 8 matmuls per group
            for k, (kind, gs, g) in enumerate(units):
                qw = qpool.tile(
                    [P, g, PER_OUT], bf16, tag=f"qw_{kind}", name=f"qw{k}"
                )
                nc.vector.tensor_scalar(
                    out=qw,
                    in0=utiles[k],
                    scalar1=float(MAGIC),
                    scalar2=-float(MAGIC),
                    op0=add,
                    op1=add,
                )
                for gp in range(g):
                    for oc in range(OC_N):
                        nc.tensor.matmul(
                            acc[:, oc, gs + gp, :],
                            lhsT=qw[:, gp, oc * P : (oc + 1) * P],
                            rhs=x2[:, gs + gp, :],
                            start=True,
                            stop=True,
                        )
                if k == N_QUARTERS - 1:
                    # sc2 prep slots in while the tail groups stream
                    nc.vector.tensor_copy(out=sc2[:, :, :, 0], in_=sc_sb)
                    nc.vector.tensor_copy(out=sc2[:, :, :, 1], in_=sc_sb)

            # ---- epilogue: out[o] = sum_{g,j} acc * sc2.  Stage A first
            # (groups < EP_SPLIT; all matmuls for those finished long ago),
            # stage B + combine after the final matmul.
            sc2f = sc2.rearrange("p oc g j -> p oc (g j)")
            es = EP_SPLIT * 2

            ysA = singles.tile([P, OC_N, es], f32)
            nc.vector.tensor_tensor(
                ysA, accf[:, :, :es], sc2f[:, :, :es], mybir.AluOpType.mult
            )
            outA = singles.tile([P, OC_N], f32)
            nc.vector.reduce_sum(
                out=outA.unsqueeze(2), in_=ysA, axis=mybir.AxisListType.X
            )

            ysB = singles.tile([P, OC_N, ng * 2 - es], f32)
            nc.vector.tensor_tensor(
                ysB, accf[:, :, es:], sc2f[:, :, es:], mybir.AluOpType.mult
            )
            outB = singles.tile([P, OC_N], f32)
            nc.vector.reduce_sum(
                out=outB.unsqueeze(2), in_=ysB, axis=mybir.AxisListType.X
            )
            out_sb = singles.tile([P, OC_N], f32)
            nc.vector.tensor_tensor(out_sb, outA, outB, add)

            # ---- transpose [128, oc] -> [oc, 128] for a contiguous store
            o_ps = psum.tile([OC_N, P], f32, tag="paux")
            nc.tensor.transpose(o_ps, out_sb, ident_p)
            outT = singles.tile([OC_N, P], f32)
            nc.vector.tensor_copy(out=outT, in_=o_ps)
            nc.sync.dma_start(out_d.rearrange("(oc p) -> oc p", p=P), outT)

    return _split_multi_waits(nc) if split_waits else nc


def make_in_maps(x, weights, scales):
    """Per-core input staging (host-side layout only)."""
    x = np.ascontiguousarray(np.asarray(x, dtype=np.float32))
    weights = np.asarray(weights, dtype=np.float32)
    scales = np.asarray(scales, dtype=np.float32)
    units = _units()
    in_maps = []
    for c in range(N_CORES):
        sl = slice(c * PER_OUT, (c + 1) * PER_OUT)
        wtc = weights[sl].T  # [in_dim, per_out]
        parts = [[], []]
        for k, (kind, gs, g) in enumerate(units):
            # [128 c, g, 1024 o]: partition-contiguous unit block
            blk = wtc[gs * P : (gs + g) * P, :].reshape(g, P, PER_OUT)
            parts[k % 2].append(blk.transpose(1, 0, 2).ravel())
        wa = np.ascontiguousarray(np.concatenate(parts[0]))
        wb = np.ascontiguousarray(np.concatenate(parts[1]))
        scc = np.ascontiguousarray(
            scales[sl].reshape(OC_N, P, NUM_GROUPS).transpose(1, 0, 2)
        )
        in_maps.append({"wa": wa, "wb": wb, "x": x, "scales": scc})
    return in_maps


def kernel(x, weights, scales):
    from concourse import bass_utils

    if "nc" not in _cache:
        _cache["nc"] = build_nc()
    nc = _cache["nc"]

    in_maps = make_in_maps(x, weights, scales)
    res = bass_utils.run_bass_kernel_spmd(nc, in_maps, core_ids=list(range(N_CORES)))
    return np.concatenate([res.results[c]["out"] for c in range(N_CORES)]).astype(
        np.float32
    )


# revision 22
# speedup vs baseline: 1.1711x; 1.1711x over previous
"""Group-quantized linear (fake int4 per-group dequant) GEMV on 8 Trainium2 cores.

Reference computation (all fp32):
    qw = round_half_even(clip(W, -8, 7))            # W in [-8, 7) so clip is identity
    out = (qw.reshape(O, 64, 128) * scales[:, :, None]).reshape(O, O) @ x

Sharding: column-parallel — each core owns a 1024-row slice of W/scales,
x replicated, outputs concatenated (per the tensor-parallel hint).

Device pipeline, built around the HBM stream (memory-bound problem):
  DMA   : TWO HW DGE queues (SP + Activation engines) stream the weights
          concurrently (~400 GB/s aggregate vs ~310 single-queue).  The
          per-core weight slice is shipped pre-packed (pure host-side
          layout) into two linear regions, one per queue.  Bulk of the
          stream moves in 2 MiB "quarter" units (4 groups; 16 KiB
          partition-contiguous descriptors — measured fastest); the last
          8 groups move as 512 KiB per-group units so the compute tail
          trails the final bytes by ~2 us instead of ~12.
  DVE   : quantize via the fp32 magic-number trick (w + 1.5*2^23) -
          1.5*2^23 == round-half-even exactly, cast to bf16 (exact for
          ints in [-8, 7]); one tensor_scalar per unit.
  PE    : per (group g, out-chunk oc) matmul acc[:, oc, g, :2] =
          qw[128c, 128o].T @ x2[128c, 2] where x2 = [x_hi | x_lo] bf16
          Dekker split of x (fp32-accurate), all accumulated in one fp32
          PSUM tile [128, 8, 64, 2] (2 banks).
  DVE   : epilogue out[o] = sum_{g,j} acc[o, oc, g, j] * scales[o, oc, g]
          with hi/lo-duplicated scales: stage A (groups < 56) is emitted
          after the tail quantizes so DVE never blocks on PE mid-tail;
          stage B (groups 56..63) + combine are 3 tiny ops after the last
          matmul.  Scales ride mid-stream, split across both queues.
  PE/DVE: transpose [128, 8] result for a contiguous output DMA
"""

import numpy as np

IN_DIM = 8192
OUT_DIM = 8192
NUM_GROUPS = 64
GROUP_SIZE = 128  # IN_DIM // NUM_GROUPS
N_CORES = 8
PER_OUT = OUT_DIM // N_CORES  # 1024
P = 128
OC_N = PER_OUT // P  # 8

MAGIC = np.float32(12582912.0)  # 1.5 * 2**23: (w + MAGIC) - MAGIC == rint(w)

GPC = 8  # groups per chunk; chunk = the 4 MiB A/B-interleaved layout block
N_CHUNKS = NUM_GROUPS // GPC  # 8
EP_SPLIT = 56  # epilogue stage-A covers groups [0, 56); stage B the last chunk

_cache = {}


def _units():
    """(group_start, n_groups) in stream order; unit k rides queue k%2.
    Chunks 0-6 move as 2 MiB half-chunk units (16 KiB partition-contiguous
    descriptors — measured fastest; the A/B halves of one chunk interleave
    at 16 KiB in HBM, which measured ~15% faster per queue than disjoint
    per-queue regions).  The last chunk moves as four staggered 1 MiB
    units so only ~2 groups of compute trail the final bytes."""
    u = []
    for ch in range(N_CHUNKS - 1):
        u.append((ch * GPC, 4))      # A: low half
        u.append((ch * GPC + 4, 4))  # B: high half
    gs = (N_CHUNKS - 1) * GPC
    u += [(gs, 2), (gs + 4, 2), (gs + 2, 2), (gs + 6, 2)]
    return u


def _split_multi_waits(nc):
    """walrus in this container accepts only ONE sync-wait per instruction;
    Tile's tail drain carries one per producer proc. Hoist extras onto
    same-engine NoOps placed immediately before — identical semantics for an
    in-order sequencer."""
    import concourse.mybir as mybir

    uid = 0
    for f in nc.m.functions:
        for blk in f.blocks:
            insts = blk.instructions
            if not any(
                i.sync_info is not None
                and i.sync_info.on_wait
                and len(i.sync_info.on_wait) > 1
                for i in insts
            ):
                continue
            new_insts = []
            for inst in insts:
                si = inst.sync_info
                if si is not None and si.on_wait and len(si.on_wait) > 1:
                    waits = list(si.on_wait)
                    for w in waits[:-1]:
                        uid += 1
                        new_insts.append(
                            mybir.InstNoOp(
                                name=f"I-waitsplit-{uid}",
                                engine=inst.engine,
                                ins=[],
                                outs=[],
                                sync_info=mybir.SyncInfo(on_wait=[w], on_update=[]),
                            )
                        )
                    inst.sync_info = mybir.SyncInfo(
                        on_wait=[waits[-1]], on_update=si.on_update
                    )
                new_insts.append(inst)
            blk.instructions = new_insts
    return nc


def build_nc(w_bufs=5, q_bufs=4, split_waits=True):
    import concourse.bass as bass
    import concourse.mybir as mybir
    import concourse.tile as tile
    from concourse.masks import make_identity

    f32 = mybir.dt.float32
    bf16 = mybir.dt.bfloat16
    add = mybir.AluOpType.add

    ng = NUM_GROUPS

    nc = bass.Bass()
    wt_d = nc.dram_tensor("wt", [IN_DIM * PER_OUT], f32, kind="ExternalInput")
    x_d = nc.dram_tensor("x", [IN_DIM], f32, kind="ExternalInput")
    sc_d = nc.dram_tensor("scales", [P, OC_N, ng], f32, kind="ExternalInput")
    out_d = nc.dram_tensor("out", [PER_OUT], f32, kind="ExternalOutput")

    units = _units()

    with tile.TileContext(nc) as tc:
        with (
            tc.tile_pool(name="singles", bufs=1) as singles,
            tc.tile_pool(name="w", bufs=w_bufs) as wpool,
            tc.tile_pool(name="q", bufs=q_bufs) as qpool,
            tc.tile_pool(name="psum", bufs=1, space="PSUM") as psum,
        ):
            # ---- x + scales first on the SP queue (tiny), then weights.
            x_nat = singles.tile([ng, GROUP_SIZE], f32)
            nc.sync.dma_start(x_nat, x_d.rearrange("(g c) -> g c", c=GROUP_SIZE))
            sc_sb = singles.tile([P, OC_N, ng], f32)
            nc.sync.dma_start(sc_sb, sc_d[:])

            # ---- weight stream: unit k on queue k%2 (A=SP, B=Act)
            wt_v = wt_d.rearrange(
                "(ch c g o) -> ch c g o", ch=N_CHUNKS, c=P, g=GPC
            )
            utiles = []
            for k, (gs, g) in enumerate(units):
                wf = wpool.tile([P, g, PER_OUT], f32, tag=f"wf{g}", name=f"wf{k}")
                eng = nc.sync if k % 2 == 0 else nc.scalar
                ch, a = gs // GPC, gs % GPC
                eng.dma_start(wf, wt_v[ch][:, a : a + g, :])
                utiles.append(wf)

            # ---- x prep: PE-transpose [ng,128] -> [128,ng], Dekker-split
            # into interleaved bf16 hi/lo [128, ng, 2].
            ident_g = singles.tile([ng, ng], f32)
            make_identity(nc, ident_g)
            ident_p = singles.tile([P, P], f32)
            make_identity(nc, ident_p)

            x_ps = psum.tile([P, ng], f32, tag="paux")
            nc.tensor.transpose(x_ps, x_nat, ident_g)
            xT = singles.tile([P, ng], f32)
            nc.vector.tensor_copy(out=xT, in_=x_ps)
            xhi = singles.tile([P, ng], bf16)
            nc.vector.tensor_copy(out=xhi, in_=xT)
            xhi32 = singles.tile([P, ng], f32)
            nc.vector.tensor_copy(out=xhi32, in_=xhi)
            xlo32 = singles.tile([P, ng], f32)
            nc.vector.tensor_tensor(xlo32, xT, xhi32, mybir.AluOpType.subtract)
            x2 = singles.tile([P, ng, 2], bf16)
            nc.vector.tensor_copy(out=x2[:, :, 0], in_=xhi)
            nc.vector.tensor_copy(out=x2[:, :, 1], in_=xlo32)

            # one fused PSUM accumulator [128, oc, g, hi/lo] (2 banks)
            acc = psum.tile([P, OC_N, ng, 2], f32, tag="pacc")
            accf = acc.rearrange("p oc g j -> p oc (g j)")

            # sc2 = scales duplicated over hi/lo, for flat epilogue APs
            sc2 = singles.tile([P, OC_N, ng, 2], f32)

            # ---- main loop: per-unit quantize + 8 matmuls per group
            for k, (gs, g) in enumerate(units):
                qw = qpool.tile(
                    [P, g, PER_OUT], bf16, tag=f"qw{g}", name=f"qw{k}"
                )
                nc.vector.tensor_scalar(
                    out=qw,
                    in0=utiles[k],
                    scalar1=float(MAGIC),
                    scalar2=-float(MAGIC),
                    op0=add,
                    op1=add,
                )
                for gp in range(g):
                    for oc in range(OC_N):
                        nc.tensor.matmul(
                            acc[:, oc, gs + gp, :],
                            lhsT=qw[:, gp, oc * P : (oc + 1) * P],
                            rhs=x2[:, gs + gp, :],
                            start=True,
                            stop=True,
                        )
                if k == 2:
                    # sc2 prep early — scales landed at the head, DVE is idle
                    nc.vector.tensor_copy(out=sc2[:, :, :, 0], in_=sc_sb)
                    nc.vector.tensor_copy(out=sc2[:, :, :, 1], in_=sc_sb)

            # ---- epilogue: out[o] = sum_{g,j} acc * sc2.  Stage A first
            # (groups < EP_SPLIT; all matmuls for those finished long ago),
            # stage B + combine after the final matmul.
            sc2f = sc2.rearrange("p oc g j -> p oc (g j)")
            es = EP_SPLIT * 2

            ysA = singles.tile([P, OC_N, es], f32)
            nc.vector.tensor_tensor(
                ysA, accf[:, :, :es], sc2f[:, :, :es], mybir.AluOpType.mult
            )
            outA = singles.tile([P, OC_N], f32)
            nc.vector.reduce_sum(
                out=outA.unsqueeze(2), in_=ysA, axis=mybir.AxisListType.X
            )

            ysB = singles.tile([P, OC_N, ng * 2 - es], f32)
            nc.vector.tensor_tensor(
                ysB, accf[:, :, es:], sc2f[:, :, es:], mybir.AluOpType.mult
            )
            outB = singles.tile([P, OC_N], f32)
            nc.vector.reduce_sum(
                out=outB.unsqueeze(2), in_=ysB, axis=mybir.AxisListType.X
            )
            out_sb = singles.tile([P, OC_N], f32)
            nc.vector.tensor_tensor(out_sb, outA, outB, add)

            # ---- transpose [128, oc] -> [oc, 128] for a contiguous store
            o_ps = psum.tile([OC_N, P], f32, tag="paux")
            nc.tensor.transpose(o_ps, out_sb, ident_p)
            outT = singles.tile([OC_N, P], f32)
            nc.vector.tensor_copy(out=outT, in_=o_ps)
            nc.sync.dma_start(out_d.rearrange("(oc p) -> oc p", p=P), outT)

    return _split_multi_waits(nc) if split_waits else nc


def make_in_maps(x, weights, scales):
    """Per-core input staging (host-side layout only)."""
    x = np.ascontiguousarray(np.asarray(x, dtype=np.float32))
    weights = np.asarray(weights, dtype=np.float32)
    scales = np.asarray(scales, dtype=np.float32)
    in_maps = []
    for c in range(N_CORES):
        sl = slice(c * PER_OUT, (c + 1) * PER_OUT)
        wtc = weights[sl].T  # [in_dim, per_out]
        # [ch, c, gp, o]: each partition's chunk data contiguous (32 KiB)
        wt = np.ascontiguousarray(
            wtc.reshape(N_CHUNKS, GPC, P, PER_OUT).transpose(0, 2, 1, 3)
        ).ravel()
        scc = np.ascontiguousarray(
            scales[sl].reshape(OC_N, P, NUM_GROUPS).transpose(1, 0, 2)
        )
        in_maps.append({"wt": wt, "x": x, "scales": scc})
    return in_maps


def kernel(x, weights, scales):
    from concourse import bass_utils

    if "nc" not in _cache:
        _cache["nc"] = build_nc()
    nc = _cache["nc"]

    in_maps = make_in_maps(x, weights, scales)
    res = bass_utils.run_bass_kernel_spmd(nc, in_maps, core_ids=list(range(N_CORES)))
    return np.concatenate([res.results[c]["out"] for c in range(N_CORES)]).astype(
        np.float32
    )
